# revision 7
# baseline (speedup 1.0000x reference)
"""GAT 2-layer kernel for Trainium2, 8 NeuronCores.

Dst-contiguous edge sharding: core c owns nodes [6250c, 6250(c+1)) and all
their incoming edges.  Per core, nodes are degree-sorted (perm) and edges
packed into a node-major slot grid (blocks of 128 nodes x Dk columns), so the
whole segment softmax is per-partition math.  Per layer: phase A computes
z/z_i/s/d for own rows on TensorE and AllGathers a 512B bf16 node-table row
[z_bf16 | s_hi | s_lo | pad] (s at ~16-bit precision via hi/lo split); phase B
dma_gathers z[src] rows (signed int16 offsets from a mid-table base, <=912
idxs + 16 positive tail per instruction), ACT computes
w=exp(lrelu(s_src+d_dst+t)) with d as per-partition bias and accumulates
softmax denominators, DVE does chained per-column weighted accumulation in
f32, then per-node normalize + relu.  Layer-2 phase A is interleaved into
layer-1 phase B so TensorE overlaps the gather stream.  Segment-max is
skipped (softmax shift invariance; exp args are O(5) here, safe in f32).

Graph-structure data (gather index stream, edge-d slot plane, slot mask) is
baked into the NEFF as Const tensors (all 8 cores' slices concatenated; each
core If(partition_id)-selects its own), so per-call host->device staging is
just attr (bf16) + the two packed weight tensors; the output is bf16 and
upcast on host.  The t-plane is computed on device as ed*ct + mask with ct
(attn_t x fc0 scalar) shipped inside the packed weight tensor, keeping
weights runtime-variable; the program cache keys on (src, dst, edge_d).
"""

import sys

sys.path.insert(0, "/opt/trn_rl_repo")

import numpy as np

N = 50000
E = 800000
F_IN = 64
D = 128
NC = 8
RANGE = N // NC            # 6250
NB = (RANGE + 127) // 128  # 49
PADN = NB * 128            # 6272
TROWS = NC * PADN          # 50176
EW = 256                   # bf16 elems: 512B row [z | s_hi | s_lo | pad]
TBASE = TROWS - 32768      # 17408
CHUNK_COLS = 7             # 896 idxs + 16 tail per gather instruction
NEG = -1.0e30
WCOLS = 2 * D + 4          # fc1.T | fc2.T | attn_s | attn_d | ct | pad

_CACHE = {}


def _host_layout(src, dst, edge_d):
    per_core = []
    for c in range(NC):
        em = (dst // RANGE) == c
        ce_src = src[em]
        ce_dst = dst[em] - c * RANGE
        deg = np.bincount(ce_dst, minlength=RANGE)
        perm = np.argsort(deg, kind="stable")
        deg_pad = np.concatenate([deg[perm], np.zeros(PADN - RANGE, np.int64)])
        per_core.append((ce_src, ce_dst, edge_d[em], deg, perm, deg_pad))

    dks = np.zeros(NB, np.int64)
    for c in range(NC):
        dks = np.maximum(dks, per_core[c][5].reshape(NB, 128).max(axis=1))
    dks = np.maximum(dks, 1)
    chunks = []
    for k in range(NB):
        j = 0
        while j < dks[k]:
            w = int(min(CHUNK_COLS, dks[k] - j))
            chunks.append((k, j, w))
            j += w
    totcols = int(dks.sum())
    colbase = np.concatenate([[0], np.cumsum(dks)])[:-1].astype(np.int64)

    row_of_node = np.zeros(N, np.int64)
    for c in range(NC):
        perm = per_core[c][4]
        pos = np.empty(RANGE, np.int64)
        pos[perm] = np.arange(RANGE)
        row_of_node[c * RANGE : (c + 1) * RANGE] = c * PADN + pos

    cores = []
    for c in range(NC):
        ce_src, ce_dst, ce_ed, deg, perm, deg_pad = per_core[c]
        order = np.argsort(ce_dst, kind="stable")
        starts = np.concatenate([[0], np.cumsum(deg)])
        slot_src = np.full((128, totcols), -1, np.int64)
        slot_ed = np.zeros((128, totcols), np.float32)
        src_sorted = ce_src[order]
        ed_sorted = ce_ed[order]
        for k in range(NB):
            b = colbase[k]
            for u in range(128):
                p = k * 128 + u
                if p >= RANGE:
                    continue
                nl = perm[p]
                d0, d1 = starts[nl], starts[nl + 1]
                w = d1 - d0
                slot_src[u, b : b + w] = src_sorted[d0:d1]
                slot_ed[u, b : b + w] = ed_sorted[d0:d1]
        # gather index stream (per chunk, 16-wrapped, +16 positive tail)
        segs = []
        for (k, j0, w) in chunks:
            cols = slot_src[:, colbase[k] + j0 : colbase[k] + j0 + w]
            flat = cols.T.reshape(-1)
            rows = np.where(flat >= 0, row_of_node[np.clip(flat, 0, N - 1)], TBASE)
            idx = (rows - TBASE).astype(np.int16)
            idx = np.concatenate([idx, np.full(16, 7, np.int16)])
            segs.append(idx.reshape(idx.size // 16, 16).T)
        idxs = np.tile(np.concatenate(segs, axis=1), (8, 1)).copy()
        mask = slot_src >= 0
        cores.append((perm, idxs, slot_ed, mask))
    return dks, chunks, totcols, colbase, cores


def _build_program(dks, chunks, totcols, colbase, cores, n_iter=1):
    import concourse.bacc as bacc
    import concourse.mybir as mybir
    import concourse.tile as tile
    from concourse.library_config import mlp
    from concourse.masks import make_identity

    f32 = mybir.dt.float32
    bf16 = mybir.dt.bfloat16
    nidx_tot = sum(128 * w + 16 for (_, _, w) in chunks)
    NIW = nidx_tot // 16
    nc = bacc.Bacc("TRN2", target_bir_lowering=False, debug=False, num_devices=NC)

    # graph-structure constants, all cores concatenated along partitions
    cidx_np = np.concatenate([cores[c][1] for c in range(NC)], axis=0)
    ced_np = np.concatenate([cores[c][2] for c in range(NC)], axis=0)
    cmask_np = np.concatenate(
        [np.where(cores[c][3], 0.0, NEG).astype(np.float32) for c in range(NC)],
        axis=0)
    cidx_t = nc.inline_tensor(np.ascontiguousarray(cidx_np), name="cidx")
    ced_t = nc.inline_tensor(np.ascontiguousarray(ced_np), name="ced")
    cmask_t = nc.inline_tensor(np.ascontiguousarray(cmask_np), name="cmask")

    attr_in = nc.dram_tensor("attr_bf", [F_IN, PADN], bf16, kind="ExternalInput")
    w1_in = nc.dram_tensor("w1", [D, WCOLS], f32, kind="ExternalInput")
    w2_in = nc.dram_tensor("w2", [D, WCOLS], f32, kind="ExternalInput")
    out2_d = nc.dram_tensor("out2", [NB, 128, D], bf16, kind="ExternalOutput")

    with tile.TileContext(nc) as tc:
        with (
            tc.tile_pool(name="const", bufs=1) as cpool,
            tc.tile_pool(name="resident", bufs=1) as rpool,
            tc.tile_pool(name="work", bufs=4) as wpool,
            tc.tile_pool(name="acc", bufs=6) as apool,
            tc.tile_pool(name="gpool", bufs=4) as gpool,
            tc.tile_pool(name="psum", bufs=2, space="PSUM") as ppool,
            tc.tile_pool(name="dram", bufs=1, space="DRAM") as dpool,
        ):
            nc.gpsimd.load_library(mlp)

            idx_sb = cpool.tile([128, NIW], mybir.dt.int16, tag="idx")
            w_sb = [cpool.tile([D, WCOLS], f32, tag=f"w{l}", name=f"wsb{l}")
                    for l in (1, 2)]
            nc.sync.dma_start(out=w_sb[0][:], in_=w1_in[:])
            nc.sync.dma_start(out=w_sb[1][:], in_=w2_in[:])
            h1_sb = cpool.tile([F_IN, PADN], f32, tag="h1")
            ident = cpool.tile([128, 128], f32, tag="ident")
            make_identity(nc, ident[:])
            tp_sb = [cpool.tile([128, totcols], f32, tag=f"tp{l}", name=f"tp{l}")
                     for l in (1, 2)]

            with tc.tile_pool(name="init", bufs=1) as ipool:
                ed_sb = ipool.tile([128, totcols], f32, tag="ed")
                mask_sb = ipool.tile([128, totcols], f32, tag="mask")
                pid = nc.partition_id()
                for c in range(NC):
                    with tc.If(pid == c):
                        nc.sync.dma_start(out=idx_sb[:],
                                          in_=cidx_t[c * 128:(c + 1) * 128, :])
                        nc.sync.dma_start(out=ed_sb[:],
                                          in_=ced_t[c * 128:(c + 1) * 128, :])
                        nc.sync.dma_start(out=mask_sb[:],
                                          in_=cmask_t[c * 128:(c + 1) * 128, :])
                attr_sb = ipool.tile([F_IN, PADN], bf16, tag="attr")
                nc.sync.dma_start(out=attr_sb[:], in_=attr_in[:])
                nc.vector.tensor_copy(h1_sb[:], attr_sb[:])
                # t-planes: ed * ct_l + mask  (ct lives in w col 2D+2)
                for li in range(2):
                    nc.vector.scalar_tensor_tensor(
                        out=tp_sb[li][:], in0=ed_sb[:],
                        scalar=w_sb[li][:, 2 * D + 2 : 2 * D + 3], in1=mask_sb[:],
                        op0=mybir.AluOpType.mult, op1=mybir.AluOpType.add)

            h2_sb = rpool.tile([D, PADN], f32, tag="h2")
            zi_sb_l = [rpool.tile([128, NB, D], f32, tag=f"zi{l}", name=f"zi{l}")
                       for l in (1, 2)]
            d_sb_l = [rpool.tile([128, NB], f32, tag=f"dcol{l}", name=f"dcol{l}")
                      for l in (1, 2)]
            agin = [None, None]
            table = [None, None]

            def fresh_tables(it):
                for li, l in enumerate((1, 2)):
                    agin[li] = dpool.tile([PADN, EW], bf16, tag=f"agin{l}_{it}",
                                          name=f"agin{l}_{it}")
                    table[li] = dpool.tile([TROWS, EW], bf16, addr_space="Shared",
                                           tag=f"tb{l}_{it}", name=f"tb{l}_{it}")

            def phase_a_tile(layer, t):
                    li = layer - 1
                    h = h1_sb if layer == 1 else h2_sb
                    w = w_sb[li]
                    K = F_IN if layer == 1 else D
                    zi_sb = zi_sb_l[li]
                    d_sb = d_sb_l[li]
                    hT = h[0:K, t * 128 : (t + 1) * 128]
                    z_ps = ppool.tile([128, D], f32, tag="z_ps", bufs=2)
                    zi_ps = ppool.tile([128, D], f32, tag="zi_ps", bufs=2)
                    zT_ps = ppool.tile([D, 128], f32, tag="zT_ps", bufs=2)
                    sd_ps = ppool.tile([128, 2], f32, tag="sd_ps", bufs=1)
                    nc.tensor.matmul(z_ps[:], lhsT=hT, rhs=w[0:K, 0:D],
                                     start=True, stop=True)
                    nc.tensor.matmul(zi_ps[:], lhsT=hT, rhs=w[0:K, D : 2 * D],
                                     start=True, stop=True)
                    nc.tensor.matmul(zT_ps[:], lhsT=w[0:K, 0:D], rhs=hT,
                                     start=True, stop=True)
                    zT_sb = wpool.tile([D, 128], f32, tag="zT_sb")
                    nc.vector.tensor_copy(zT_sb[:], zT_ps[:])
                    nc.tensor.matmul(sd_ps[:], lhsT=zT_sb[:],
                                     rhs=w[0:D, 2 * D : 2 * D + 2],
                                     start=True, stop=True)
                    asm = wpool.tile([128, EW], bf16, tag="asm")
                    nc.vector.tensor_copy(asm[:, 0:D], z_ps[:])
                    nc.vector.tensor_copy(asm[:, D : D + 1], sd_ps[:, 0:1])
                    shi_f = wpool.tile([128, 1], f32, tag="shi_f")
                    nc.vector.tensor_copy(shi_f[:], asm[:, D : D + 1])
                    slo = wpool.tile([128, 1], f32, tag="slo")
                    nc.vector.tensor_tensor(slo[:], sd_ps[:, 0:1], shi_f[:],
                                            op=mybir.AluOpType.subtract)
                    nc.vector.tensor_copy(asm[:, D + 1 : D + 2], slo[:])
                    nc.vector.tensor_copy(d_sb[:, t : t + 1], sd_ps[:, 1:2])
                    nc.vector.tensor_copy(zi_sb[:, t, :], zi_ps[:])
                    nc.sync.dma_start(out=agin[li][t * 128 : (t + 1) * 128, :],
                                      in_=asm[:])

            def phase_a(layer):
                for t in range(NB):
                    phase_a_tile(layer, t)

            def all_gather(layer):
                li = layer - 1
                nc.gpsimd.collective_compute(
                    "AllGather", mybir.AluOpType.bypass,
                    replica_groups=[list(range(NC))],
                    ins=[agin[li][:]], outs=[table[li][:]])

            def phase_b(layer):
                li = layer - 1
                tb = table[li]
                tp = tp_sb[li]
                zi_sb = zi_sb_l[li]
                d_sb = d_sb_l[li]
                znb = den = None
                idx_off = 0
                for (k, j0, w) in chunks:
                    ni = 128 * w + 16
                    ncols = (ni + 127) // 128
                    g = gpool.tile([128, CHUNK_COLS + 1, EW], bf16, tag="g")
                    nc.gpsimd.dma_gather(
                        out_ap=g[:, 0:ncols, :], in_ap=tb[TBASE:, :],
                        idxs_ap=idx_sb[:, idx_off : idx_off + ni // 16],
                        num_idxs=ni, num_idxs_reg=ni, elem_size=EW)
                    idx_off += ni // 16
                    b = int(colbase[k])
                    ssum = wpool.tile([128, CHUNK_COLS], f32, tag="ssum")
                    nc.vector.scalar_tensor_tensor(
                        out=ssum[:, 0:w], in0=g[:, 0:w, D], scalar=1.0,
                        in1=g[:, 0:w, D + 1],
                        op0=mybir.AluOpType.mult, op1=mybir.AluOpType.add)
                    u_t = wpool.tile([128, CHUNK_COLS], f32, tag="u_t")
                    nc.vector.scalar_tensor_tensor(
                        out=u_t[:, 0:w], in0=tp[:, b + j0 : b + j0 + w],
                        scalar=1.0, in1=ssum[:, 0:w],
                        op0=mybir.AluOpType.mult, op1=mybir.AluOpType.add)
                    lr_t = wpool.tile([128, CHUNK_COLS], f32, tag="lr_t")
                    nc.scalar.activation(lr_t[:, 0:w], u_t[:, 0:w],
                                         mybir.ActivationFunctionType.Lrelu,
                                         bias=d_sb[:, k : k + 1], scale=1.0,
                                         alpha=0.01)
                    first = j0 == 0
                    if first:
                        znb = apool.tile([128, D], f32, tag="znb")
                        den = apool.tile([128, 1], f32, tag="den")
                    w_t = wpool.tile([128, CHUNK_COLS], f32, tag="w_t")
                    dpart = den if first else wpool.tile([128, 1], f32, tag="dpart")
                    nc.scalar.activation(w_t[:, 0:w], lr_t[:, 0:w],
                                         mybir.ActivationFunctionType.Exp,
                                         accum_out=dpart[:])
                    # chained weighted accumulation: znb += g[:,j,:] * w[:,j]
                    for j in range(w):
                        if first and j == 0:
                            nc.vector.tensor_scalar(
                                out=znb[:], in0=g[:, j, 0:D],
                                scalar1=w_t[:, j : j + 1], scalar2=None,
                                op0=mybir.AluOpType.mult)
                        else:
                            nc.vector.scalar_tensor_tensor(
                                out=znb[:], in0=g[:, j, 0:D],
                                scalar=w_t[:, j : j + 1], in1=znb[:],
                                op0=mybir.AluOpType.mult,
                                op1=mybir.AluOpType.add)
                    if not first:
                        nc.vector.tensor_tensor(den[:], den[:], dpart[:],
                                                op=mybir.AluOpType.add)
                    if j0 + w == dks[k]:
                        rec = wpool.tile([128, 1], f32, tag="rec")
                        nc.vector.tensor_scalar_max(den[:], den[:], 1e-9)
                        nc.vector.reciprocal(rec[:], den[:])
                        comb = wpool.tile([128, D], f32, tag="comb")
                        nc.vector.scalar_tensor_tensor(
                            out=comb[:], in0=znb[:], scalar=rec[:],
                            in1=zi_sb[:, k, :], op0=mybir.AluOpType.mult,
                            op1=mybir.AluOpType.add)
                        if layer == 1:
                            o = wpool.tile([128, D], f32, tag="o")
                            nc.scalar.activation(o[:], comb[:],
                                                 mybir.ActivationFunctionType.Relu)
                            oT = ppool.tile([D, 128], f32, tag="oT", bufs=1)
                            nc.tensor.transpose(oT[:], o[:], ident[:])
                            nc.vector.tensor_copy(
                                h2_sb[:, k * 128 : (k + 1) * 128], oT[:])
                            phase_a_tile(2, k)
                        else:
                            o = wpool.tile([128, D], bf16, tag="obf")
                            nc.scalar.activation(o[:], comb[:],
                                                 mybir.ActivationFunctionType.Relu)
                            nc.sync.dma_start(out=out2_d[k], in_=o[:])

            for it in range(n_iter):
                fresh_tables(it)
                phase_a(1)
                all_gather(1)
                phase_b(1)
                all_gather(2)
                phase_b(2)

    nc.compile()
    return nc


def _prepare(src, dst, edge_d):
    key = (src.tobytes(), dst.tobytes(), edge_d.tobytes())
    if _CACHE.get("key") != key:
        dks, chunks, totcols, colbase, cores = _host_layout(src, dst, edge_d)
        prog = _build_program(dks, chunks, totcols, colbase, cores)
        _CACHE.clear()
        _CACHE.update(key=key, dks=dks, chunks=chunks, totcols=totcols,
                      colbase=colbase, cores=cores, prog=prog)
    return (_CACHE["dks"], _CACHE["chunks"], _CACHE["totcols"],
            _CACHE["colbase"], _CACHE["cores"], _CACHE["prog"])


def build_in_maps(attr, edge_d, src, dst,
                  fc0_w1, fc1_w1, fc2_w1, attn_w1,
                  fc0_w2, fc1_w2, fc2_w2, attn_w2):
    import ml_dtypes
    bf = ml_dtypes.bfloat16
    attr = np.asarray(attr, np.float32)
    edge_d = np.asarray(edge_d, np.float32).reshape(-1)
    src = np.asarray(src, np.int64)
    dst = np.asarray(dst, np.int64)
    dks, chunks, totcols, colbase, cores, prog = _prepare(src, dst, edge_d)

    def wpack(fc1, fc2, attn, fc0, K):
        w = np.zeros((D, WCOLS), np.float32)
        w[0:K, 0:D] = np.asarray(fc1, np.float32).T
        w[0:K, D : 2 * D] = np.asarray(fc2, np.float32).T
        a = np.asarray(attn, np.float32)[0]
        w[0:D, 2 * D] = a[0:D]
        w[0:D, 2 * D + 1] = a[D : 2 * D]
        ct = float(a[2 * D]) * float(np.asarray(fc0, np.float32)[0, 0])
        w[0:D, 2 * D + 2] = ct
        return w

    w1p = wpack(fc1_w1, fc2_w1, attn_w1, fc0_w1, F_IN)
    w2p = wpack(fc1_w2, fc2_w2, attn_w2, fc0_w2, D)

    in_maps = []
    for c in range(NC):
        perm = cores[c][0]
        ap = np.zeros((PADN, F_IN), np.float32)
        ap[:RANGE] = attr[c * RANGE : (c + 1) * RANGE][perm]
        in_maps.append({"attr_bf": np.ascontiguousarray(ap.T).astype(bf),
                        "w1": w1p, "w2": w2p})
    return prog, in_maps, cores


def kernel(attr, edge_d, src, dst,
           fc0_w1, fc1_w1, fc2_w1, attn_w1,
           fc0_w2, fc1_w2, fc2_w2, attn_w2, _trace=False):
    prog, in_maps, cores = build_in_maps(
        attr, edge_d, src, dst, fc0_w1, fc1_w1, fc2_w1, attn_w1,
        fc0_w2, fc1_w2, fc2_w2, attn_w2)
    res = run_bass_kernel_spmd_cached(prog, in_maps, trace=_trace)
    out = np.zeros((N, D), np.float32)
    for c in range(NC):
        perm = cores[c][0]
        o = res.results[c]["out2"].reshape(PADN, D)[:RANGE].astype(np.float32)
        out[c * RANGE + perm] = o
    if _trace:
        return out, res
    return out


def run_bass_kernel_spmd_cached(prog, in_maps, trace=False):
    from concourse.bass_utils import run_bass_kernel_spmd
    last = None
    for attempt in range(3):
        try:
            return run_bass_kernel_spmd(prog, in_maps,
                                        core_ids=list(range(NC)), trace=trace)
        except Exception as e:  # transient NRT_EXEC_UNIT_UNRECOVERABLE flakes
            last = e
            import time as _t
            _t.sleep(5)
    raise last
